# revision 25
# baseline (speedup 1.0000x reference)
"""DotProductDistributionHead kernel for Trainium2 (Bass/Tile), 8-core data-parallel.

Computation (per reference):
    h = gelu(x @ W_mu + b_mu)            # (B, D) with erf gelu
    logits[b, n] = h[b] . emb_table[candidates[b, n]] + mu_bias[candidates[b, n]]

Sharding: x/candidates split along batch across 8 cores; W_mu / b_mu /
emb_table replicated per core (each core's copy lands in its own HBM
stack, so gather bandwidth scales with cores).

v2 design vs the v1 baseline (kernel_v1_baseline.py):
  * emb_table is converted to fp16 on the host: halves gather DMA bytes and
    enables the DVE 16-bit 2x mode. fp16 keeps ~2e-4 relative error (the
    values are ~N(0, 0.02^2); accumulation stays fp32).
  * SWDGE int16 gather indices cover the 100K-row table via NW overlapping
    32768-row value windows. Each row's 200 candidates are split across
    windows by a greedy-left balanced assignment (windows overlap, so most
    values have a choice of 2 windows) -> per-(row,window) load is nearly
    uniform and slot-grid padding drops from ~40% to ~10-15%.
  * One dma_gather per window (4-6 total vs 32): the Pool engine pays a
    measured ~6us fixed cost per gather instruction.
  * Dot product: fp16 tensor_tensor mult (2x mode) + an in-place fold tree
    (d: 128->64->32->16 via 2x fp16 adds) + one 16-wide tensor_reduce
    (tensor_reduce only has a 1x uop, so folding first is ~1.5x faster).

mu_bias is all-zeros per the problem spec; a host-side fallback adds it if
a nonzero vector is ever passed.
"""

import os

import numpy as np

import concourse.bacc as bacc
import concourse.bass as bass
import concourse.tile as tile
from concourse import mybir
from concourse.bass_utils import run_bass_kernel_spmd

B, N, D, V = 4096, 200, 128, 100000
NCORES = 8
B_LOC = B // NCORES          # 512 batch rows per core
NBLK = B_LOC // 128          # 4 blocks of 128 rows
WIN = 32768                  # int16 index reach per gather instruction

TRACE = False
LAST_RESULTS = None
ACT_FUNC = "Gelu"
REPEATS = 1                  # bench-only: emit the main loop R times (slope timing)
FOLD_STOP = 16               # fold d down to this width, then tensor_reduce
# The HWDGE dynamic-offset gather (_kernel_body_indirect) hangs on real
# hardware (NRT INTERNAL error) although it passes CoreSim — keep the SWDGE
# window-gather path as the default.
USE_INDIRECT = os.environ.get("KERNEL_INDIRECT", "0") == "1"
# single_packet=True also hangs real HW (NRT INTERNAL) — must stay False.
SINGLE_PACKET = os.environ.get("KERNEL_SINGLE_PACKET", "0") == "1"

_f32 = mybir.dt.float32
_f16 = mybir.dt.float16
_i16 = mybir.dt.int16

CONST_COLS = D + B_LOC + D   # [W | xT | b_mu replicated]

_program_cache = {}


# --------------------------------------------------------------------------
# window assignment (host)
# --------------------------------------------------------------------------

def _window_bases(nw):
    """nw overlapping windows of width WIN covering [0, V)."""
    if nw == 1:
        return [0]
    step = (V - WIN) / (nw - 1)
    return [int(round(i * step)) for i in range(nw)]


def _assign_windows(sc, nw):
    """sc: [R, N] per-row ascending candidate values. Greedy-left balanced
    assignment to nw overlapping windows under a global per-window cap G.
    Returns (bases, loads [R, nw], G) or None if infeasible (never for
    nw>=4 with these sizes)."""
    R = sc.shape[0]
    bases = _window_bases(nw)
    lo = np.empty((R, nw + 1), dtype=np.int64)   # first idx reachable by wins >= k
    hi = np.empty((R, nw), dtype=np.int64)       # first idx beyond window k
    for k in range(nw):
        lo[:, k] = np.sum(sc < bases[k], axis=1)
        hi[:, k] = np.sum(sc < min(bases[k] + WIN, V + 1), axis=1)
    lo[:, nw] = N
    if not np.all(hi[:, nw - 1] == N):
        return None  # last window must reach V

    Gmin = (N + nw - 1) // nw
    for G in range(Gmin, N + 1):
        done = np.zeros(R, dtype=np.int64)
        loads = np.zeros((R, nw), dtype=np.int64)
        ok = True
        for k in range(nw):
            n_k = np.minimum(G, hi[:, k] - done)
            n_k = np.maximum(n_k, 0)
            # after window k, everything below base_{k+1} must be assigned
            if np.any(done + n_k < lo[:, k + 1]):
                ok = False
                break
            loads[:, k] = n_k
            done = done + n_k
        if ok and np.all(done == N):
            return bases, loads, G
    return None


# --------------------------------------------------------------------------
# device program
# --------------------------------------------------------------------------

def _kernel_body(tc, consts, gidx, emb, out, g_list, out_cols, tot_cols):
    nc = tc.nc
    gelu = getattr(mybir.ActivationFunctionType, ACT_FUNC)
    nw = len(g_list)
    g_max = max(g_list)
    total_words = sum(NBLK * g * 128 // 16 for g in g_list)

    with (
        tc.tile_pool(name="const", bufs=1) as cpool,
        tc.tile_pool(name="psum", bufs=2, space="PSUM") as ppool,
        tc.tile_pool(name="outs", bufs=2) as outpool,
        tc.tile_pool(name="gather", bufs=12) as gpool,
    ):
        c_sb = cpool.tile([128, CONST_COLS], _f32)
        nc.sync.dma_start(c_sb[:], consts[:, :])
        W_sb = c_sb[:, 0:D]
        xT_sb = c_sb[:, D : D + B_LOC]
        bias_sb = c_sb[:, D + B_LOC : D + B_LOC + D]

        gidx_sb = cpool.tile([128, total_words], _i16)
        nc.sync.dma_start(gidx_sb[:], gidx[:, :])

        # h[b, d] in fp16 for all 512 local rows: block c at h_sb[:, c*D:(c+1)*D]
        h_sb = cpool.tile([128, NBLK * D], _f16)
        for c in range(NBLK):
            ps = ppool.tile([128, D], _f32)
            nc.tensor.matmul(
                out=ps[:], lhsT=xT_sb[:, c * 128 : (c + 1) * 128], rhs=W_sb,
                start=True, stop=True,
            )
            nc.vector.tensor_tensor(
                out=ps[:], in0=ps[:], in1=bias_sb, op=mybir.AluOpType.add
            )
            nc.scalar.activation(out=h_sb[:, c * D : (c + 1) * D], in_=ps[:], func=gelu)

        lim_win = int(os.environ.get("KERNEL_NWIN", 10**6))
        skip_gather = bool(os.environ.get("KERNEL_SKIP_GATHER"))
        skip_compute = bool(os.environ.get("KERNEL_SKIP_COMPUTE"))
        for _rep in range(REPEATS):
            logits_sb = outpool.tile([128, NBLK * tot_cols], _f32, tag="ltile")
            if lim_win < nw or skip_compute:
                nc.vector.memset(logits_sb[:], 0.0)
            word_off = 0
            qrr = 0
            for k in range(nw):
                g_k = g_list[k]
                nwords = NBLK * g_k * 128 // 16
                if k >= lim_win:
                    word_off += nwords
                    continue
                # chunk = 1 block (6912 idxs -> 433 descs/engine): two chunks
                # per queue fit the ~1024-desc SWDGE ring, so desc-gen never
                # stalls mid-instruction, and the 4-deep gather pool overlaps
                # gather N+1..N+3 with compute N.
                for b0 in range(0, NBLK, 1):
                    nb = min(1, NBLK - b0)
                    cnum = nb * g_k * 128
                    cwords = cnum // 16
                    woff_c = word_off + b0 * g_k * 128 // 16
                    G = gpool.tile([128, g_max * D], _f16, tag="gtile")
                    if not skip_gather:
                        nc.gpsimd.dma_gather(
                            out_ap=G[:, : nb * g_k * D].rearrange(
                                "p (t d) -> p t d", d=D
                            ),
                            in_ap=emb[_BASES_HACK[k] :, :],
                            idxs_ap=gidx_sb[:, woff_c : woff_c + cwords],
                            num_idxs=cnum,
                            num_idxs_reg=cnum,
                            elem_size=D,
                            single_packet=SINGLE_PACKET,
                            queue_num=qrr % 4,
                        )
                        qrr += 1
                    if skip_compute:
                        continue
                    for ci in range(nb):
                        c = b0 + ci
                        blk = G[:, ci * g_k * D : (ci + 1) * g_k * D]
                        G3 = blk.rearrange("p (t d) -> p t d", d=D)
                        h_bc = (
                            h_sb[:, c * D : (c + 1) * D]
                            .unsqueeze(1)
                            .to_broadcast([128, g_k, D])
                        )
                        nc.vector.tensor_tensor(out=G3, in0=G3, in1=h_bc,
                                                op=mybir.AluOpType.mult)
                        w = D
                        while w > FOLD_STOP:
                            h_w = w // 2
                            nc.vector.tensor_tensor(
                                out=G3[:, :, 0:h_w], in0=G3[:, :, 0:h_w],
                                in1=G3[:, :, h_w:w], op=mybir.AluOpType.add,
                            )
                            w = h_w
                        col = c * tot_cols + out_cols[k]
                        nc.vector.tensor_reduce(
                            out=logits_sb[:, col : col + g_k],
                            in_=G3[:, :, 0:w],
                            axis=mybir.AxisListType.X,
                            op=mybir.AluOpType.add,
                        )
                word_off += nwords
            nc.sync.dma_start(out[:, :], logits_sb[:])


def _kernel_body_indirect(tc, consts, cidx, emb, out):
    """HWDGE dynamic-offset gather: full-range uint32 indices, no windows,
    no padding. One indirect DMA per (block, half): out[p, j, :] =
    emb[cidx[p, j]]; cidx[p, c*N + j] = candidates[c*128 + p, j] (original
    candidate order, so assembly is a plain reshape)."""
    nc = tc.nc
    gelu = getattr(mybir.ActivationFunctionType, ACT_FUNC)
    HALF = N // 2

    with (
        tc.tile_pool(name="const", bufs=1) as cpool,
        tc.tile_pool(name="psum", bufs=2, space="PSUM") as ppool,
        tc.tile_pool(name="outs", bufs=2) as outpool,
        tc.tile_pool(name="gather", bufs=4) as gpool,
    ):
        c_sb = cpool.tile([128, CONST_COLS], _f32)
        nc.sync.dma_start(c_sb[:], consts[:, :])
        W_sb = c_sb[:, 0:D]
        xT_sb = c_sb[:, D : D + B_LOC]
        bias_sb = c_sb[:, D + B_LOC : D + B_LOC + D]

        cidx_sb = cpool.tile([128, NBLK * N], mybir.dt.uint32)
        nc.sync.dma_start(cidx_sb[:], cidx[:, :])

        h_sb = cpool.tile([128, NBLK * D], _f16)
        for c in range(NBLK):
            ps = ppool.tile([128, D], _f32)
            nc.tensor.matmul(
                out=ps[:], lhsT=xT_sb[:, c * 128 : (c + 1) * 128], rhs=W_sb,
                start=True, stop=True,
            )
            nc.vector.tensor_tensor(
                out=ps[:], in0=ps[:], in1=bias_sb, op=mybir.AluOpType.add
            )
            nc.scalar.activation(out=h_sb[:, c * D : (c + 1) * D], in_=ps[:], func=gelu)

        skip_gather = bool(os.environ.get("KERNEL_SKIP_GATHER"))
        skip_compute = bool(os.environ.get("KERNEL_SKIP_COMPUTE"))
        for _rep in range(REPEATS):
            logits_sb = outpool.tile([128, NBLK * N], _f32, tag="ltile")
            if skip_compute:
                nc.vector.memset(logits_sb[:], 0.0)
            for c in range(NBLK):
                for s in range(2):
                    G = gpool.tile([128, HALF * D], _f16, tag="gtile")
                    G3 = G.rearrange("p (t d) -> p t d", d=D)
                    if not skip_gather:
                        nc.gpsimd.indirect_dma_start(
                            out=G3,
                            out_offset=None,
                            in_=emb[:, :],
                            in_offset=bass.IndirectOffsetOnAxis(
                                ap=cidx_sb[:, c * N + s * HALF : c * N + (s + 1) * HALF],
                                axis=0,
                            ),
                        )
                    if skip_compute:
                        continue
                    h_bc = (
                        h_sb[:, c * D : (c + 1) * D]
                        .unsqueeze(1)
                        .to_broadcast([128, HALF, D])
                    )
                    nc.vector.tensor_tensor(out=G3, in0=G3, in1=h_bc,
                                            op=mybir.AluOpType.mult)
                    w = D
                    while w > FOLD_STOP:
                        h_w = w // 2
                        nc.vector.tensor_tensor(
                            out=G3[:, :, 0:h_w], in0=G3[:, :, 0:h_w],
                            in1=G3[:, :, h_w:w], op=mybir.AluOpType.add,
                        )
                        w = h_w
                    col = c * N + s * HALF
                    nc.vector.tensor_reduce(
                        out=logits_sb[:, col : col + HALF],
                        in_=G3[:, :, 0:w],
                        axis=mybir.AxisListType.X,
                        op=mybir.AluOpType.add,
                    )
            nc.sync.dma_start(out[:, :], logits_sb[:])


def _build_program_indirect():
    key = ("indirect", ACT_FUNC, REPEATS, FOLD_STOP)
    if key in _program_cache:
        return _program_cache[key]
    nc = bacc.Bacc(
        "TRN2",
        target_bir_lowering=False,
        debug=False,
        enable_asserts=False,
        num_devices=NCORES,
        num_swdge_queues=4,
    )
    consts = nc.dram_tensor("consts", (128, CONST_COLS), _f32, kind="ExternalInput").ap()
    cidx = nc.dram_tensor("cidx", (128, NBLK * N), mybir.dt.uint32, kind="ExternalInput").ap()
    emb = nc.dram_tensor("emb", (V, D), _f16, kind="ExternalInput").ap()
    out = nc.dram_tensor("out", (128, NBLK * N), _f32, kind="ExternalOutput").ap()
    with tile.TileContext(nc) as tc:
        _kernel_body_indirect(tc, consts, cidx, emb, out)
    nc.finalize()
    _program_cache[key] = nc
    return nc


def prepare_indirect(x, candidates, W_mu, b_mu, mu_bias, emb_table):
    x = np.asarray(x, dtype=np.float32)
    candidates = np.asarray(candidates).astype(np.int64)
    W_mu = np.ascontiguousarray(np.asarray(W_mu, dtype=np.float32))
    b_mu = np.asarray(b_mu, dtype=np.float32)
    emb16 = np.ascontiguousarray(
        np.asarray(emb_table, dtype=np.float32).astype(np.float16)
    )
    nc = _build_program_indirect()
    bias_tile = np.broadcast_to(b_mu.reshape(1, D), (128, D))
    in_maps = []
    for core in range(NCORES):
        sl = slice(core * B_LOC, (core + 1) * B_LOC)
        consts = np.concatenate([W_mu, x[sl].T, bias_tile], axis=1)
        # cidx[p, c*N + j] = candidates[core*B_LOC + c*128 + p, j]
        cc = candidates[sl].reshape(NBLK, 128, N).transpose(1, 0, 2).reshape(128, NBLK * N)
        in_maps.append(
            {
                "consts": np.ascontiguousarray(consts, dtype=np.float32),
                "cidx": np.ascontiguousarray(cc.astype(np.uint32)),
                "emb": emb16,
            }
        )
    return nc, in_maps


def assemble_indirect(results):
    logits = np.zeros((B, N), dtype=np.float32)
    for core in range(len(results)):
        oc = results[core]["out"]  # [128, NBLK*N]
        rs = slice(core * B_LOC, (core + 1) * B_LOC)
        logits[rs] = oc.reshape(128, NBLK, N).transpose(1, 0, 2).reshape(B_LOC, N)
    return logits


_BASES_HACK = None  # set by _build_program; window base row offsets into emb


def _build_program(bases, g_list, out_cols, tot_cols):
    global _BASES_HACK
    key = (tuple(bases), tuple(g_list), ACT_FUNC, REPEATS, FOLD_STOP, SINGLE_PACKET)
    if key in _program_cache:
        return _program_cache[key]
    nc = bacc.Bacc(
        "TRN2",
        target_bir_lowering=False,
        debug=False,
        enable_asserts=False,
        num_devices=NCORES,
        num_swdge_queues=4,
    )
    total_words = sum(NBLK * g * 128 // 16 for g in g_list)
    consts = nc.dram_tensor("consts", (128, CONST_COLS), _f32, kind="ExternalInput").ap()
    gidx = nc.dram_tensor("gidx", (128, total_words), _i16, kind="ExternalInput").ap()
    emb = nc.dram_tensor("emb", (V, D), _f16, kind="ExternalInput").ap()
    out = nc.dram_tensor("out", (128, NBLK * tot_cols), _f32, kind="ExternalOutput").ap()
    _BASES_HACK = bases
    with tile.TileContext(nc) as tc:
        _kernel_body(tc, consts, gidx, emb, out, g_list, out_cols, tot_cols)
    nc.finalize()
    _program_cache[key] = nc
    return nc


# --------------------------------------------------------------------------
# host packing
# --------------------------------------------------------------------------

def prepare(x, candidates, W_mu, b_mu, mu_bias, emb_table):
    x = np.asarray(x, dtype=np.float32)
    candidates = np.asarray(candidates).astype(np.int64)
    W_mu = np.ascontiguousarray(np.asarray(W_mu, dtype=np.float32))
    b_mu = np.asarray(b_mu, dtype=np.float32)
    emb16 = np.ascontiguousarray(np.asarray(emb_table, dtype=np.float32).astype(np.float16))

    order = np.argsort(candidates, axis=1, kind="stable")    # [B, N]
    sc = np.take_along_axis(candidates, order, axis=1)       # sorted values

    best = None
    for nw in (6,):  # finer windows pipeline better: 5888-idx gathers = 368
                     # descs/engine, ~2.8 per SWDGE ring vs 2.3 at nw=5
        res = _assign_windows(sc, nw)
        if res is None:
            continue
        bases, loads, G = res
        g_list = [int(loads[:, k].max()) for k in range(nw)]
        tot = sum(g_list)
        if best is None or tot < best[0]:
            best = (tot, bases, loads, g_list)
    assert best is not None, "window assignment failed"
    tot_cols, bases, loads, g_list = best
    nw = len(g_list)
    out_cols = np.concatenate([[0], np.cumsum(g_list)])[:nw].tolist()

    # col_map: for batch row b, sorted-candidate j -> output column within the
    # row's block: col = off_k + s  (s = j - start of window k's run)
    starts = np.concatenate(
        [np.zeros((B, 1), dtype=np.int64), np.cumsum(loads, axis=1)], axis=1
    )  # [B, nw+1]; run k = sorted positions [starts[k], starts[k+1])
    col_sorted = np.empty((B, N), dtype=np.int64)
    win_of = np.empty((B, N), dtype=np.int64)
    for k in range(nw):
        s0 = starts[:, k]
        s1 = starts[:, k + 1]
        j = np.arange(N)[None, :]
        m = (j >= s0[:, None]) & (j < s1[:, None])
        col_sorted[m] = (out_cols[k] + (j - s0[:, None]))[m]
        win_of[m] = k
    # map back to original candidate order: col_map[b, order[b,j]] = col_sorted[b,j]
    col_map = np.empty((B, N), dtype=np.int64)
    np.put_along_axis(col_map, order, col_sorted, axis=1)

    # per-core packed gather indices
    gidx_tiles = []
    total_words = sum(NBLK * g * 128 // 16 for g in g_list)
    for core in range(NCORES):
        rs = slice(core * B_LOC, (core + 1) * B_LOC)
        sc_c = sc[rs]
        starts_c = starts[rs]
        words = np.empty((16, total_words), dtype=np.uint16)
        woff = 0
        for k in range(nw):
            g_k = g_list[k]
            vals = np.zeros((B_LOC, g_k), dtype=np.uint16)  # pad -> idx 0
            ld = (starts_c[:, k + 1] - starts_c[:, k]).astype(np.int64)
            # gather run k of each row
            j = np.arange(g_k)[None, :]
            src = starts_c[:, k][:, None] + j
            m = j < ld[:, None]
            picked = np.take_along_axis(sc_c, np.minimum(src, N - 1), axis=1)
            vals[m] = (picked - bases[k]).astype(np.uint16)[m]
            # slot grid: flat j = (c*g_k + t)*128 + p ; vals[c*128+p, t]
            v = vals.reshape(NBLK, 128, g_k).transpose(0, 2, 1).reshape(-1)
            nwords = v.size // 16
            words[:, woff : woff + nwords] = v.reshape(nwords, 16).T
            woff += nwords
        gidx_tiles.append(np.tile(words, (8, 1)).view(np.int16))

    # sanity: indices in range for every window
    for k in range(nw):
        m = win_of == k
        if m.any():
            assert int(sc[m].min()) >= bases[k], (k, int(sc[m].min()), bases[k])
            assert int(sc[m].max()) - bases[k] <= 32767, (k, int(sc[m].max()), bases[k])

    nc = _build_program(tuple(bases), g_list, out_cols, tot_cols)

    bias_tile = np.broadcast_to(b_mu.reshape(1, D), (128, D))
    in_maps = []
    for core in range(NCORES):
        sl = slice(core * B_LOC, (core + 1) * B_LOC)
        consts = np.concatenate([W_mu, x[sl].T, bias_tile], axis=1)
        in_maps.append(
            {
                "consts": np.ascontiguousarray(consts, dtype=np.float32),
                "gidx": np.ascontiguousarray(gidx_tiles[core]),
                "emb": emb16,
            }
        )
    return nc, in_maps, col_map, tot_cols


def assemble(results, col_map, tot_cols):
    logits = np.zeros((B, N), dtype=np.float32)
    for core in range(len(results)):
        oc = results[core]["out"]  # [128, NBLK * tot_cols]
        oc = oc.reshape(128, NBLK, tot_cols).transpose(1, 0, 2).reshape(B_LOC, tot_cols)
        rs = slice(core * B_LOC, (core + 1) * B_LOC)
        logits[rs] = np.take_along_axis(oc, col_map[rs], axis=1)
    return logits


def kernel(x, candidates, W_mu, b_mu, mu_bias, emb_table):
    global LAST_RESULTS
    candidates = np.asarray(candidates).astype(np.int64)
    mu_bias = np.asarray(mu_bias, dtype=np.float32)
    ncores_run = int(os.environ.get("KERNEL_CORES", NCORES))
    if USE_INDIRECT:
        nc, in_maps = prepare_indirect(
            x, candidates, W_mu, b_mu, mu_bias, emb_table
        )
        res = run_bass_kernel_spmd(
            nc, in_maps[:ncores_run], core_ids=list(range(ncores_run)), trace=TRACE
        )
        LAST_RESULTS = res
        logits = assemble_indirect(res.results)
    else:
        nc, in_maps, col_map, tot_cols = prepare(
            x, candidates, W_mu, b_mu, mu_bias, emb_table
        )
        res = run_bass_kernel_spmd(
            nc, in_maps[:ncores_run], core_ids=list(range(ncores_run)), trace=TRACE
        )
        LAST_RESULTS = res
        logits = assemble(res.results, col_map, tot_cols)
    if np.any(mu_bias):
        logits = logits + mu_bias[candidates]
    return np.ascontiguousarray(logits.astype(np.float32))


# revision 29
# speedup vs baseline: 1.5735x; 1.5735x over previous
"""DotProductDistributionHead kernel for Trainium2 (Bass/Tile), 8-core data-parallel.

Computation (per reference):
    h = gelu(x @ W_mu + b_mu)            # (B, D) with erf gelu
    logits[b, n] = h[b] . emb_table[candidates[b, n]] + mu_bias[candidates[b, n]]

Sharding: x/candidates split along batch across 8 cores; W_mu / b_mu /
emb_table replicated per core (each core's copy lands in its own HBM
stack, so gather bandwidth scales with cores).

v2 design vs the v1 baseline (kernel_v1_baseline.py):
  * emb_table is converted to fp16 on the host: halves gather DMA bytes and
    enables the DVE 16-bit 2x mode. fp16 keeps ~2e-4 relative error (the
    values are ~N(0, 0.02^2); accumulation stays fp32).
  * SWDGE int16 gather indices cover the 100K-row table via NW overlapping
    32768-row value windows. Each row's 200 candidates are split across
    windows by a greedy-left balanced assignment (windows overlap, so most
    values have a choice of 2 windows) -> per-(row,window) load is nearly
    uniform and slot-grid padding drops from ~40% to ~10-15%.
  * One dma_gather per window (4-6 total vs 32): the Pool engine pays a
    measured ~6us fixed cost per gather instruction.
  * Dot product: fp16 tensor_tensor mult (2x mode) + an in-place fold tree
    (d: 128->64->32->16 via 2x fp16 adds) + one 16-wide tensor_reduce
    (tensor_reduce only has a 1x uop, so folding first is ~1.5x faster).

mu_bias is all-zeros per the problem spec; a host-side fallback adds it if
a nonzero vector is ever passed.
"""

import os

import numpy as np

import concourse.bacc as bacc
import concourse.bass as bass
import concourse.tile as tile
from concourse import mybir
from concourse.bass_utils import run_bass_kernel_spmd

B, N, D, V = 4096, 200, 128, 100000
NCORES = 8
B_LOC = B // NCORES          # 512 batch rows per core
NBLK = B_LOC // 128          # 4 blocks of 128 rows
WIN = 32768                  # int16 index reach per gather instruction

TRACE = False
LAST_RESULTS = None
ACT_FUNC = "Gelu"
REPEATS = 1                  # bench-only: emit the main loop R times (slope timing)
FOLD_STOP = 16               # fold d down to this width, then tensor_reduce
# The HWDGE dynamic-offset gather (_kernel_body_indirect) hangs on real
# hardware (NRT INTERNAL error) although it passes CoreSim — keep the SWDGE
# window-gather path as the default.
USE_INDIRECT = os.environ.get("KERNEL_INDIRECT", "0") == "1"
# single_packet=True also hangs real HW (NRT INTERNAL) — must stay False.
SINGLE_PACKET = os.environ.get("KERNEL_SINGLE_PACKET", "0") == "1"

_f32 = mybir.dt.float32
_f16 = mybir.dt.float16
_i16 = mybir.dt.int16

CONST_COLS = D + B_LOC + D   # [W | xT | b_mu replicated]

_program_cache = {}


# --------------------------------------------------------------------------
# window assignment (host)
# --------------------------------------------------------------------------

def _window_bases(nw):
    """nw overlapping windows of width WIN covering [0, V)."""
    if nw == 1:
        return [0]
    step = (V - WIN) / (nw - 1)
    return [int(round(i * step)) for i in range(nw)]


def _assign_windows(sc, nw):
    """sc: [R, N] per-row ascending candidate values. Greedy-left balanced
    assignment to nw overlapping windows under a global per-window cap G.
    Returns (bases, loads [R, nw], G) or None if infeasible (never for
    nw>=4 with these sizes)."""
    R = sc.shape[0]
    bases = _window_bases(nw)
    lo = np.empty((R, nw + 1), dtype=np.int64)   # first idx reachable by wins >= k
    hi = np.empty((R, nw), dtype=np.int64)       # first idx beyond window k
    for k in range(nw):
        lo[:, k] = np.sum(sc < bases[k], axis=1)
        hi[:, k] = np.sum(sc < min(bases[k] + WIN, V + 1), axis=1)
    lo[:, nw] = N
    if not np.all(hi[:, nw - 1] == N):
        return None  # last window must reach V

    Gmin = (N + nw - 1) // nw
    for G in range(Gmin, N + 1):
        done = np.zeros(R, dtype=np.int64)
        loads = np.zeros((R, nw), dtype=np.int64)
        ok = True
        for k in range(nw):
            avail = np.maximum(np.minimum(G, hi[:, k] - done), 0)
            # balanced target instead of fill-to-cap: later windows' maxima
            # (and their slot grids) shrink, cutting gather padding
            target = -((done - N) // (nw - k))  # ceil((N-done)/(nw-k))
            # look-ahead floor: values below base_{j+1} must fit in windows
            # <= j under cap G
            force = lo[:, k + 1] - done
            for j in range(k + 1, nw):
                force = np.maximum(force, lo[:, j + 1] - done - (j - k) * G)
            n_k = np.minimum(avail, np.maximum(target, force))
            if np.any(n_k < force):
                ok = False
                break
            loads[:, k] = n_k
            done = done + n_k
        if ok and np.all(done == N):
            return bases, loads, G
    return None


# --------------------------------------------------------------------------
# device program
# --------------------------------------------------------------------------

def _kernel_body(tc, consts, gidx, emb, out, g_list, out_cols, tot_cols):
    nc = tc.nc
    gelu = getattr(mybir.ActivationFunctionType, ACT_FUNC)
    nw = len(g_list)
    g_max = max(g_list)
    total_words = sum(NBLK * g * 128 // 16 for g in g_list)

    with (
        tc.tile_pool(name="const", bufs=1) as cpool,
        tc.tile_pool(name="psum", bufs=2, space="PSUM") as ppool,
        tc.tile_pool(name="outs", bufs=2) as outpool,
        tc.tile_pool(name="gather", bufs=10) as gpool,
    ):
        c_sb = cpool.tile([128, CONST_COLS], _f32)
        nc.sync.dma_start(c_sb[:], consts[:, :])
        W_sb = c_sb[:, 0:D]
        xT_sb = c_sb[:, D : D + B_LOC]
        bias_sb = c_sb[:, D + B_LOC : D + B_LOC + D]

        gidx_sb = cpool.tile([128, total_words], _i16)
        nc.sync.dma_start(gidx_sb[:], gidx[:, :])

        # h[b, d] in fp16 for all 512 local rows: block c at h_sb[:, c*D:(c+1)*D]
        h_sb = cpool.tile([128, NBLK * D], _f16)
        for c in range(NBLK):
            ps = ppool.tile([128, D], _f32)
            nc.tensor.matmul(
                out=ps[:], lhsT=xT_sb[:, c * 128 : (c + 1) * 128], rhs=W_sb,
                start=True, stop=True,
            )
            nc.vector.tensor_tensor(
                out=ps[:], in0=ps[:], in1=bias_sb, op=mybir.AluOpType.add
            )
            nc.scalar.activation(out=h_sb[:, c * D : (c + 1) * D], in_=ps[:], func=gelu)

        lim_win = int(os.environ.get("KERNEL_NWIN", 10**6))
        skip_gather = bool(os.environ.get("KERNEL_SKIP_GATHER"))
        skip_compute = bool(os.environ.get("KERNEL_SKIP_COMPUTE"))
        for _rep in range(REPEATS):
            logits_sb = outpool.tile([128, NBLK * tot_cols], _f32, tag="ltile")
            if lim_win < nw or skip_compute:
                nc.vector.memset(logits_sb[:], 0.0)
            word_off = 0
            qrr = 0
            for k in range(nw):
                g_k = g_list[k]
                nwords = NBLK * g_k * 128 // 16
                if k >= lim_win:
                    word_off += nwords
                    continue
                # chunk = 1 block (6912 idxs -> 433 descs/engine): two chunks
                # per queue fit the ~1024-desc SWDGE ring, so desc-gen never
                # stalls mid-instruction, and the 4-deep gather pool overlaps
                # gather N+1..N+3 with compute N.
                for b0 in range(0, NBLK, 1):
                    nb = min(1, NBLK - b0)
                    cnum = nb * g_k * 128
                    cwords = cnum // 16
                    woff_c = word_off + b0 * g_k * 128 // 16
                    G = gpool.tile([128, g_max * D], _f16, tag="gtile")
                    if not skip_gather:
                        nc.gpsimd.dma_gather(
                            out_ap=G[:, : nb * g_k * D].rearrange(
                                "p (t d) -> p t d", d=D
                            ),
                            in_ap=emb[_BASES_HACK[k] :, :],
                            idxs_ap=gidx_sb[:, woff_c : woff_c + cwords],
                            num_idxs=cnum,
                            num_idxs_reg=cnum,
                            elem_size=D,
                            single_packet=SINGLE_PACKET,
                            queue_num=qrr % 4,
                        )
                        qrr += 1
                    if skip_compute:
                        continue
                    for ci in range(nb):
                        c = b0 + ci
                        blk = G[:, ci * g_k * D : (ci + 1) * g_k * D]
                        G3 = blk.rearrange("p (t d) -> p t d", d=D)
                        h_bc = (
                            h_sb[:, c * D : (c + 1) * D]
                            .unsqueeze(1)
                            .to_broadcast([128, g_k, D])
                        )
                        nc.vector.tensor_tensor(out=G3, in0=G3, in1=h_bc,
                                                op=mybir.AluOpType.mult)
                        w = D
                        while w > FOLD_STOP:
                            h_w = w // 2
                            nc.vector.tensor_tensor(
                                out=G3[:, :, 0:h_w], in0=G3[:, :, 0:h_w],
                                in1=G3[:, :, h_w:w], op=mybir.AluOpType.add,
                            )
                            w = h_w
                        col = c * tot_cols + out_cols[k]
                        nc.vector.tensor_reduce(
                            out=logits_sb[:, col : col + g_k],
                            in_=G3[:, :, 0:w],
                            axis=mybir.AxisListType.X,
                            op=mybir.AluOpType.add,
                        )
                word_off += nwords
            nc.sync.dma_start(out[:, :], logits_sb[:])


def _kernel_body_indirect(tc, consts, cidx, emb, out):
    """HWDGE dynamic-offset gather: full-range uint32 indices, no windows,
    no padding. One indirect DMA per (block, half): out[p, j, :] =
    emb[cidx[p, j]]; cidx[p, c*N + j] = candidates[c*128 + p, j] (original
    candidate order, so assembly is a plain reshape)."""
    nc = tc.nc
    gelu = getattr(mybir.ActivationFunctionType, ACT_FUNC)
    HALF = N // 2

    with (
        tc.tile_pool(name="const", bufs=1) as cpool,
        tc.tile_pool(name="psum", bufs=2, space="PSUM") as ppool,
        tc.tile_pool(name="outs", bufs=2) as outpool,
        tc.tile_pool(name="gather", bufs=4) as gpool,
    ):
        c_sb = cpool.tile([128, CONST_COLS], _f32)
        nc.sync.dma_start(c_sb[:], consts[:, :])
        W_sb = c_sb[:, 0:D]
        xT_sb = c_sb[:, D : D + B_LOC]
        bias_sb = c_sb[:, D + B_LOC : D + B_LOC + D]

        cidx_sb = cpool.tile([128, NBLK * N], mybir.dt.uint32)
        nc.sync.dma_start(cidx_sb[:], cidx[:, :])

        h_sb = cpool.tile([128, NBLK * D], _f16)
        for c in range(NBLK):
            ps = ppool.tile([128, D], _f32)
            nc.tensor.matmul(
                out=ps[:], lhsT=xT_sb[:, c * 128 : (c + 1) * 128], rhs=W_sb,
                start=True, stop=True,
            )
            nc.vector.tensor_tensor(
                out=ps[:], in0=ps[:], in1=bias_sb, op=mybir.AluOpType.add
            )
            nc.scalar.activation(out=h_sb[:, c * D : (c + 1) * D], in_=ps[:], func=gelu)

        skip_gather = bool(os.environ.get("KERNEL_SKIP_GATHER"))
        skip_compute = bool(os.environ.get("KERNEL_SKIP_COMPUTE"))
        for _rep in range(REPEATS):
            logits_sb = outpool.tile([128, NBLK * N], _f32, tag="ltile")
            if skip_compute:
                nc.vector.memset(logits_sb[:], 0.0)
            for c in range(NBLK):
                for s in range(2):
                    G = gpool.tile([128, HALF * D], _f16, tag="gtile")
                    G3 = G.rearrange("p (t d) -> p t d", d=D)
                    if not skip_gather:
                        nc.gpsimd.indirect_dma_start(
                            out=G3,
                            out_offset=None,
                            in_=emb[:, :],
                            in_offset=bass.IndirectOffsetOnAxis(
                                ap=cidx_sb[:, c * N + s * HALF : c * N + (s + 1) * HALF],
                                axis=0,
                            ),
                        )
                    if skip_compute:
                        continue
                    h_bc = (
                        h_sb[:, c * D : (c + 1) * D]
                        .unsqueeze(1)
                        .to_broadcast([128, HALF, D])
                    )
                    nc.vector.tensor_tensor(out=G3, in0=G3, in1=h_bc,
                                            op=mybir.AluOpType.mult)
                    w = D
                    while w > FOLD_STOP:
                        h_w = w // 2
                        nc.vector.tensor_tensor(
                            out=G3[:, :, 0:h_w], in0=G3[:, :, 0:h_w],
                            in1=G3[:, :, h_w:w], op=mybir.AluOpType.add,
                        )
                        w = h_w
                    col = c * N + s * HALF
                    nc.vector.tensor_reduce(
                        out=logits_sb[:, col : col + HALF],
                        in_=G3[:, :, 0:w],
                        axis=mybir.AxisListType.X,
                        op=mybir.AluOpType.add,
                    )
            nc.sync.dma_start(out[:, :], logits_sb[:])


def _build_program_indirect():
    key = ("indirect", ACT_FUNC, REPEATS, FOLD_STOP)
    if key in _program_cache:
        return _program_cache[key]
    nc = bacc.Bacc(
        "TRN2",
        target_bir_lowering=False,
        debug=False,
        enable_asserts=False,
        num_devices=NCORES,
        num_swdge_queues=4,
    )
    consts = nc.dram_tensor("consts", (128, CONST_COLS), _f32, kind="ExternalInput").ap()
    cidx = nc.dram_tensor("cidx", (128, NBLK * N), mybir.dt.uint32, kind="ExternalInput").ap()
    emb = nc.dram_tensor("emb", (V, D), _f16, kind="ExternalInput").ap()
    out = nc.dram_tensor("out", (128, NBLK * N), _f32, kind="ExternalOutput").ap()
    with tile.TileContext(nc) as tc:
        _kernel_body_indirect(tc, consts, cidx, emb, out)
    nc.finalize()
    _program_cache[key] = nc
    return nc


def prepare_indirect(x, candidates, W_mu, b_mu, mu_bias, emb_table):
    x = np.asarray(x, dtype=np.float32)
    candidates = np.asarray(candidates).astype(np.int64)
    W_mu = np.ascontiguousarray(np.asarray(W_mu, dtype=np.float32))
    b_mu = np.asarray(b_mu, dtype=np.float32)
    emb16 = np.ascontiguousarray(
        np.asarray(emb_table, dtype=np.float32).astype(np.float16)
    )
    nc = _build_program_indirect()
    bias_tile = np.broadcast_to(b_mu.reshape(1, D), (128, D))
    in_maps = []
    for core in range(NCORES):
        sl = slice(core * B_LOC, (core + 1) * B_LOC)
        consts = np.concatenate([W_mu, x[sl].T, bias_tile], axis=1)
        # cidx[p, c*N + j] = candidates[core*B_LOC + c*128 + p, j]
        cc = candidates[sl].reshape(NBLK, 128, N).transpose(1, 0, 2).reshape(128, NBLK * N)
        in_maps.append(
            {
                "consts": np.ascontiguousarray(consts, dtype=np.float32),
                "cidx": np.ascontiguousarray(cc.astype(np.uint32)),
                "emb": emb16,
            }
        )
    return nc, in_maps


def assemble_indirect(results):
    logits = np.zeros((B, N), dtype=np.float32)
    for core in range(len(results)):
        oc = results[core]["out"]  # [128, NBLK*N]
        rs = slice(core * B_LOC, (core + 1) * B_LOC)
        logits[rs] = oc.reshape(128, NBLK, N).transpose(1, 0, 2).reshape(B_LOC, N)
    return logits


_BASES_HACK = None  # set by _build_program; window base row offsets into emb


def _build_program(bases, g_list, out_cols, tot_cols):
    global _BASES_HACK
    key = (tuple(bases), tuple(g_list), ACT_FUNC, REPEATS, FOLD_STOP, SINGLE_PACKET)
    if key in _program_cache:
        return _program_cache[key]
    nc = bacc.Bacc(
        "TRN2",
        target_bir_lowering=False,
        debug=False,
        enable_asserts=False,
        num_devices=NCORES,
        num_swdge_queues=4,
    )
    total_words = sum(NBLK * g * 128 // 16 for g in g_list)
    consts = nc.dram_tensor("consts", (128, CONST_COLS), _f32, kind="ExternalInput").ap()
    gidx = nc.dram_tensor("gidx", (128, total_words), _i16, kind="ExternalInput").ap()
    emb = nc.dram_tensor("emb", (V, D), _f16, kind="ExternalInput").ap()
    out = nc.dram_tensor("out", (128, NBLK * tot_cols), _f32, kind="ExternalOutput").ap()
    _BASES_HACK = bases
    with tile.TileContext(nc) as tc:
        _kernel_body(tc, consts, gidx, emb, out, g_list, out_cols, tot_cols)
    nc.finalize()
    _program_cache[key] = nc
    return nc


# --------------------------------------------------------------------------
# host packing
# --------------------------------------------------------------------------

def prepare(x, candidates, W_mu, b_mu, mu_bias, emb_table):
    x = np.asarray(x, dtype=np.float32)
    candidates = np.asarray(candidates).astype(np.int64)
    W_mu = np.ascontiguousarray(np.asarray(W_mu, dtype=np.float32))
    b_mu = np.asarray(b_mu, dtype=np.float32)
    emb16 = np.ascontiguousarray(np.asarray(emb_table, dtype=np.float32).astype(np.float16))

    order = np.argsort(candidates, axis=1, kind="stable")    # [B, N]
    sc = np.take_along_axis(candidates, order, axis=1)       # sorted values

    best = None
    for nw in (4, 5):  # nw=6's 24 instructions regressed on HW (757us)
        res = _assign_windows(sc, nw)
        if res is None:
            continue
        bases, loads, G = res
        g_list = [int(loads[:, k].max()) for k in range(nw)]
        tot = sum(g_list)
        if best is None or tot < best[0]:
            best = (tot, bases, loads, g_list)
    assert best is not None, "window assignment failed"
    tot_cols, bases, loads, g_list = best
    nw = len(g_list)
    out_cols = np.concatenate([[0], np.cumsum(g_list)])[:nw].tolist()

    # col_map: for batch row b, sorted-candidate j -> output column within the
    # row's block: col = off_k + s  (s = j - start of window k's run)
    starts = np.concatenate(
        [np.zeros((B, 1), dtype=np.int64), np.cumsum(loads, axis=1)], axis=1
    )  # [B, nw+1]; run k = sorted positions [starts[k], starts[k+1])
    col_sorted = np.empty((B, N), dtype=np.int64)
    win_of = np.empty((B, N), dtype=np.int64)
    for k in range(nw):
        s0 = starts[:, k]
        s1 = starts[:, k + 1]
        j = np.arange(N)[None, :]
        m = (j >= s0[:, None]) & (j < s1[:, None])
        col_sorted[m] = (out_cols[k] + (j - s0[:, None]))[m]
        win_of[m] = k
    # map back to original candidate order: col_map[b, order[b,j]] = col_sorted[b,j]
    col_map = np.empty((B, N), dtype=np.int64)
    np.put_along_axis(col_map, order, col_sorted, axis=1)

    # per-core packed gather indices
    gidx_tiles = []
    total_words = sum(NBLK * g * 128 // 16 for g in g_list)
    for core in range(NCORES):
        rs = slice(core * B_LOC, (core + 1) * B_LOC)
        sc_c = sc[rs]
        starts_c = starts[rs]
        words = np.empty((16, total_words), dtype=np.uint16)
        woff = 0
        for k in range(nw):
            g_k = g_list[k]
            vals = np.zeros((B_LOC, g_k), dtype=np.uint16)  # pad -> idx 0
            ld = (starts_c[:, k + 1] - starts_c[:, k]).astype(np.int64)
            # gather run k of each row
            j = np.arange(g_k)[None, :]
            src = starts_c[:, k][:, None] + j
            m = j < ld[:, None]
            picked = np.take_along_axis(sc_c, np.minimum(src, N - 1), axis=1)
            vals[m] = (picked - bases[k]).astype(np.uint16)[m]
            # slot grid: flat j = (c*g_k + t)*128 + p ; vals[c*128+p, t]
            v = vals.reshape(NBLK, 128, g_k).transpose(0, 2, 1).reshape(-1)
            nwords = v.size // 16
            words[:, woff : woff + nwords] = v.reshape(nwords, 16).T
            woff += nwords
        gidx_tiles.append(np.tile(words, (8, 1)).view(np.int16))

    # sanity: indices in range for every window
    for k in range(nw):
        m = win_of == k
        if m.any():
            assert int(sc[m].min()) >= bases[k], (k, int(sc[m].min()), bases[k])
            assert int(sc[m].max()) - bases[k] <= 32767, (k, int(sc[m].max()), bases[k])

    nc = _build_program(tuple(bases), g_list, out_cols, tot_cols)

    bias_tile = np.broadcast_to(b_mu.reshape(1, D), (128, D))
    in_maps = []
    for core in range(NCORES):
        sl = slice(core * B_LOC, (core + 1) * B_LOC)
        consts = np.concatenate([W_mu, x[sl].T, bias_tile], axis=1)
        in_maps.append(
            {
                "consts": np.ascontiguousarray(consts, dtype=np.float32),
                "gidx": np.ascontiguousarray(gidx_tiles[core]),
                "emb": emb16,
            }
        )
    return nc, in_maps, col_map, tot_cols


def assemble(results, col_map, tot_cols):
    logits = np.zeros((B, N), dtype=np.float32)
    for core in range(len(results)):
        oc = results[core]["out"]  # [128, NBLK * tot_cols]
        oc = oc.reshape(128, NBLK, tot_cols).transpose(1, 0, 2).reshape(B_LOC, tot_cols)
        rs = slice(core * B_LOC, (core + 1) * B_LOC)
        logits[rs] = np.take_along_axis(oc, col_map[rs], axis=1)
    return logits


def kernel(x, candidates, W_mu, b_mu, mu_bias, emb_table):
    global LAST_RESULTS
    candidates = np.asarray(candidates).astype(np.int64)
    mu_bias = np.asarray(mu_bias, dtype=np.float32)
    ncores_run = int(os.environ.get("KERNEL_CORES", NCORES))
    if USE_INDIRECT:
        nc, in_maps = prepare_indirect(
            x, candidates, W_mu, b_mu, mu_bias, emb_table
        )
        res = run_bass_kernel_spmd(
            nc, in_maps[:ncores_run], core_ids=list(range(ncores_run)), trace=TRACE
        )
        LAST_RESULTS = res
        logits = assemble_indirect(res.results)
    else:
        nc, in_maps, col_map, tot_cols = prepare(
            x, candidates, W_mu, b_mu, mu_bias, emb_table
        )
        res = run_bass_kernel_spmd(
            nc, in_maps[:ncores_run], core_ids=list(range(ncores_run)), trace=TRACE
        )
        LAST_RESULTS = res
        logits = assemble(res.results, col_map, tot_cols)
    if np.any(mu_bias):
        logits = logits + mu_bias[candidates]
    return np.ascontiguousarray(logits.astype(np.float32))


# revision 31
# speedup vs baseline: 1.5787x; 1.0033x over previous
"""DotProductDistributionHead kernel for Trainium2 (Bass/Tile), 8-core data-parallel.

Computation (per reference):
    h = gelu(x @ W_mu + b_mu)            # (B, D) with erf gelu
    logits[b, n] = h[b] . emb_table[candidates[b, n]] + mu_bias[candidates[b, n]]

Sharding: x/candidates split along batch across 8 cores; W_mu / b_mu /
emb_table replicated per core (each core's copy lands in its own HBM
stack, so gather bandwidth scales with cores).

v2 design vs the v1 baseline (kernel_v1_baseline.py):
  * emb_table is converted to fp16 on the host: halves gather DMA bytes and
    enables the DVE 16-bit 2x mode. fp16 keeps ~2e-4 relative error (the
    values are ~N(0, 0.02^2); accumulation stays fp32).
  * SWDGE int16 gather indices cover the 100K-row table via NW overlapping
    32768-row value windows. Each row's 200 candidates are split across
    windows by a greedy-left balanced assignment (windows overlap, so most
    values have a choice of 2 windows) -> per-(row,window) load is nearly
    uniform and slot-grid padding drops from ~40% to ~10-15%.
  * One dma_gather per window (4-6 total vs 32): the Pool engine pays a
    measured ~6us fixed cost per gather instruction.
  * Dot product: fp16 tensor_tensor mult (2x mode) + an in-place fold tree
    (d: 128->64->32->16 via 2x fp16 adds) + one 16-wide tensor_reduce
    (tensor_reduce only has a 1x uop, so folding first is ~1.5x faster).

mu_bias is all-zeros per the problem spec; a host-side fallback adds it if
a nonzero vector is ever passed.
"""

import os

import numpy as np

import concourse.bacc as bacc
import concourse.bass as bass
import concourse.tile as tile
from concourse import mybir
from concourse.bass_utils import run_bass_kernel_spmd

B, N, D, V = 4096, 200, 128, 100000
NCORES = 8
B_LOC = B // NCORES          # 512 batch rows per core
NBLK = B_LOC // 128          # 4 blocks of 128 rows
WIN = 32768                  # int16 index reach per gather instruction

TRACE = False
LAST_RESULTS = None
ACT_FUNC = "Gelu"
REPEATS = 1                  # bench-only: emit the main loop R times (slope timing)
FOLD_STOP = 8                # fold d down to this width, then tensor_reduce
# The HWDGE dynamic-offset gather (_kernel_body_indirect) hangs on real
# hardware (NRT INTERNAL error) although it passes CoreSim — keep the SWDGE
# window-gather path as the default.
USE_INDIRECT = os.environ.get("KERNEL_INDIRECT", "0") == "1"
# single_packet=True also hangs real HW (NRT INTERNAL) — must stay False.
SINGLE_PACKET = os.environ.get("KERNEL_SINGLE_PACKET", "0") == "1"

_f32 = mybir.dt.float32
_f16 = mybir.dt.float16
_i16 = mybir.dt.int16

CONST_COLS = D + B_LOC + D   # [W | xT | b_mu replicated]

_program_cache = {}


# --------------------------------------------------------------------------
# window assignment (host)
# --------------------------------------------------------------------------

def _window_bases(nw):
    """nw overlapping windows of width WIN covering [0, V)."""
    if nw == 1:
        return [0]
    step = (V - WIN) / (nw - 1)
    return [int(round(i * step)) for i in range(nw)]


def _assign_windows(sc, nw):
    """sc: [R, N] per-row ascending candidate values. Greedy-left balanced
    assignment to nw overlapping windows under a global per-window cap G.
    Returns (bases, loads [R, nw], G) or None if infeasible (never for
    nw>=4 with these sizes)."""
    R = sc.shape[0]
    bases = _window_bases(nw)
    lo = np.empty((R, nw + 1), dtype=np.int64)   # first idx reachable by wins >= k
    hi = np.empty((R, nw), dtype=np.int64)       # first idx beyond window k
    for k in range(nw):
        lo[:, k] = np.sum(sc < bases[k], axis=1)
        hi[:, k] = np.sum(sc < min(bases[k] + WIN, V + 1), axis=1)
    lo[:, nw] = N
    if not np.all(hi[:, nw - 1] == N):
        return None  # last window must reach V

    Gmin = (N + nw - 1) // nw
    for G in range(Gmin, N + 1):
        done = np.zeros(R, dtype=np.int64)
        loads = np.zeros((R, nw), dtype=np.int64)
        ok = True
        for k in range(nw):
            avail = np.maximum(np.minimum(G, hi[:, k] - done), 0)
            # balanced target instead of fill-to-cap: later windows' maxima
            # (and their slot grids) shrink, cutting gather padding
            target = -((done - N) // (nw - k))  # ceil((N-done)/(nw-k))
            # look-ahead floor: values below base_{j+1} must fit in windows
            # <= j under cap G
            force = lo[:, k + 1] - done
            for j in range(k + 1, nw):
                force = np.maximum(force, lo[:, j + 1] - done - (j - k) * G)
            n_k = np.minimum(avail, np.maximum(target, force))
            if np.any(n_k < force):
                ok = False
                break
            loads[:, k] = n_k
            done = done + n_k
        if ok and np.all(done == N):
            return bases, loads, G
    return None


# --------------------------------------------------------------------------
# device program
# --------------------------------------------------------------------------

def _kernel_body(tc, consts, gidx, emb, out, g_list, out_cols, tot_cols):
    nc = tc.nc
    gelu = getattr(mybir.ActivationFunctionType, ACT_FUNC)
    nw = len(g_list)
    g_max = max(g_list)
    total_words = sum(NBLK * g * 128 // 16 for g in g_list)

    with (
        tc.tile_pool(name="const", bufs=1) as cpool,
        tc.tile_pool(name="psum", bufs=2, space="PSUM") as ppool,
        tc.tile_pool(name="outs", bufs=2) as outpool,
        tc.tile_pool(name="gather", bufs=11) as gpool,
    ):
        c_sb = cpool.tile([128, CONST_COLS], _f32)
        nc.sync.dma_start(c_sb[:], consts[:, :])
        W_sb = c_sb[:, 0:D]
        xT_sb = c_sb[:, D : D + B_LOC]
        bias_sb = c_sb[:, D + B_LOC : D + B_LOC + D]

        gidx_sb = cpool.tile([128, total_words], _i16)
        nc.sync.dma_start(gidx_sb[:], gidx[:, :])

        # h[b, d] in fp16 for all 512 local rows: block c at h_sb[:, c*D:(c+1)*D]
        h_sb = cpool.tile([128, NBLK * D], _f16)
        for c in range(NBLK):
            ps = ppool.tile([128, D], _f32)
            nc.tensor.matmul(
                out=ps[:], lhsT=xT_sb[:, c * 128 : (c + 1) * 128], rhs=W_sb,
                start=True, stop=True,
            )
            nc.vector.tensor_tensor(
                out=ps[:], in0=ps[:], in1=bias_sb, op=mybir.AluOpType.add
            )
            nc.scalar.activation(out=h_sb[:, c * D : (c + 1) * D], in_=ps[:], func=gelu)

        lim_win = int(os.environ.get("KERNEL_NWIN", 10**6))
        skip_gather = bool(os.environ.get("KERNEL_SKIP_GATHER"))
        skip_compute = bool(os.environ.get("KERNEL_SKIP_COMPUTE"))
        for _rep in range(REPEATS):
            logits_sb = outpool.tile([128, NBLK * tot_cols], _f32, tag="ltile")
            if lim_win < nw or skip_compute:
                nc.vector.memset(logits_sb[:], 0.0)
            word_off = 0
            qrr = 0
            for k in range(nw):
                g_k = g_list[k]
                nwords = NBLK * g_k * 128 // 16
                if k >= lim_win:
                    word_off += nwords
                    continue
                # chunk = 1 block (6912 idxs -> 433 descs/engine): two chunks
                # per queue fit the ~1024-desc SWDGE ring, so desc-gen never
                # stalls mid-instruction, and the 4-deep gather pool overlaps
                # gather N+1..N+3 with compute N.
                for b0 in range(0, NBLK, 1):
                    nb = min(1, NBLK - b0)
                    cnum = nb * g_k * 128
                    cwords = cnum // 16
                    woff_c = word_off + b0 * g_k * 128 // 16
                    G = gpool.tile([128, g_max * D], _f16, tag="gtile")
                    if not skip_gather:
                        nc.gpsimd.dma_gather(
                            out_ap=G[:, : nb * g_k * D].rearrange(
                                "p (t d) -> p t d", d=D
                            ),
                            in_ap=emb[_BASES_HACK[k] :, :],
                            idxs_ap=gidx_sb[:, woff_c : woff_c + cwords],
                            num_idxs=cnum,
                            num_idxs_reg=cnum,
                            elem_size=D,
                            single_packet=SINGLE_PACKET,
                            queue_num=qrr % 4,
                        )
                        qrr += 1
                    if skip_compute:
                        continue
                    for ci in range(nb):
                        c = b0 + ci
                        blk = G[:, ci * g_k * D : (ci + 1) * g_k * D]
                        G3 = blk.rearrange("p (t d) -> p t d", d=D)
                        h_bc = (
                            h_sb[:, c * D : (c + 1) * D]
                            .unsqueeze(1)
                            .to_broadcast([128, g_k, D])
                        )
                        nc.vector.tensor_tensor(out=G3, in0=G3, in1=h_bc,
                                                op=mybir.AluOpType.mult)
                        w = D
                        while w > FOLD_STOP:
                            h_w = w // 2
                            nc.vector.tensor_tensor(
                                out=G3[:, :, 0:h_w], in0=G3[:, :, 0:h_w],
                                in1=G3[:, :, h_w:w], op=mybir.AluOpType.add,
                            )
                            w = h_w
                        col = c * tot_cols + out_cols[k]
                        nc.vector.tensor_reduce(
                            out=logits_sb[:, col : col + g_k],
                            in_=G3[:, :, 0:w],
                            axis=mybir.AxisListType.X,
                            op=mybir.AluOpType.add,
                        )
                word_off += nwords
            nc.sync.dma_start(out[:, :], logits_sb[:])


def _kernel_body_indirect(tc, consts, cidx, emb, out):
    """HWDGE dynamic-offset gather: full-range uint32 indices, no windows,
    no padding. One indirect DMA per (block, half): out[p, j, :] =
    emb[cidx[p, j]]; cidx[p, c*N + j] = candidates[c*128 + p, j] (original
    candidate order, so assembly is a plain reshape)."""
    nc = tc.nc
    gelu = getattr(mybir.ActivationFunctionType, ACT_FUNC)
    HALF = N // 2

    with (
        tc.tile_pool(name="const", bufs=1) as cpool,
        tc.tile_pool(name="psum", bufs=2, space="PSUM") as ppool,
        tc.tile_pool(name="outs", bufs=2) as outpool,
        tc.tile_pool(name="gather", bufs=4) as gpool,
    ):
        c_sb = cpool.tile([128, CONST_COLS], _f32)
        nc.sync.dma_start(c_sb[:], consts[:, :])
        W_sb = c_sb[:, 0:D]
        xT_sb = c_sb[:, D : D + B_LOC]
        bias_sb = c_sb[:, D + B_LOC : D + B_LOC + D]

        cidx_sb = cpool.tile([128, NBLK * N], mybir.dt.uint32)
        nc.sync.dma_start(cidx_sb[:], cidx[:, :])

        h_sb = cpool.tile([128, NBLK * D], _f16)
        for c in range(NBLK):
            ps = ppool.tile([128, D], _f32)
            nc.tensor.matmul(
                out=ps[:], lhsT=xT_sb[:, c * 128 : (c + 1) * 128], rhs=W_sb,
                start=True, stop=True,
            )
            nc.vector.tensor_tensor(
                out=ps[:], in0=ps[:], in1=bias_sb, op=mybir.AluOpType.add
            )
            nc.scalar.activation(out=h_sb[:, c * D : (c + 1) * D], in_=ps[:], func=gelu)

        skip_gather = bool(os.environ.get("KERNEL_SKIP_GATHER"))
        skip_compute = bool(os.environ.get("KERNEL_SKIP_COMPUTE"))
        for _rep in range(REPEATS):
            logits_sb = outpool.tile([128, NBLK * N], _f32, tag="ltile")
            if skip_compute:
                nc.vector.memset(logits_sb[:], 0.0)
            for c in range(NBLK):
                for s in range(2):
                    G = gpool.tile([128, HALF * D], _f16, tag="gtile")
                    G3 = G.rearrange("p (t d) -> p t d", d=D)
                    if not skip_gather:
                        nc.gpsimd.indirect_dma_start(
                            out=G3,
                            out_offset=None,
                            in_=emb[:, :],
                            in_offset=bass.IndirectOffsetOnAxis(
                                ap=cidx_sb[:, c * N + s * HALF : c * N + (s + 1) * HALF],
                                axis=0,
                            ),
                        )
                    if skip_compute:
                        continue
                    h_bc = (
                        h_sb[:, c * D : (c + 1) * D]
                        .unsqueeze(1)
                        .to_broadcast([128, HALF, D])
                    )
                    nc.vector.tensor_tensor(out=G3, in0=G3, in1=h_bc,
                                            op=mybir.AluOpType.mult)
                    w = D
                    while w > FOLD_STOP:
                        h_w = w // 2
                        nc.vector.tensor_tensor(
                            out=G3[:, :, 0:h_w], in0=G3[:, :, 0:h_w],
                            in1=G3[:, :, h_w:w], op=mybir.AluOpType.add,
                        )
                        w = h_w
                    col = c * N + s * HALF
                    nc.vector.tensor_reduce(
                        out=logits_sb[:, col : col + HALF],
                        in_=G3[:, :, 0:w],
                        axis=mybir.AxisListType.X,
                        op=mybir.AluOpType.add,
                    )
            nc.sync.dma_start(out[:, :], logits_sb[:])


def _build_program_indirect():
    key = ("indirect", ACT_FUNC, REPEATS, FOLD_STOP)
    if key in _program_cache:
        return _program_cache[key]
    nc = bacc.Bacc(
        "TRN2",
        target_bir_lowering=False,
        debug=False,
        enable_asserts=False,
        num_devices=NCORES,
        num_swdge_queues=4,
    )
    consts = nc.dram_tensor("consts", (128, CONST_COLS), _f32, kind="ExternalInput").ap()
    cidx = nc.dram_tensor("cidx", (128, NBLK * N), mybir.dt.uint32, kind="ExternalInput").ap()
    emb = nc.dram_tensor("emb", (V, D), _f16, kind="ExternalInput").ap()
    out = nc.dram_tensor("out", (128, NBLK * N), _f32, kind="ExternalOutput").ap()
    with tile.TileContext(nc) as tc:
        _kernel_body_indirect(tc, consts, cidx, emb, out)
    nc.finalize()
    _program_cache[key] = nc
    return nc


def prepare_indirect(x, candidates, W_mu, b_mu, mu_bias, emb_table):
    x = np.asarray(x, dtype=np.float32)
    candidates = np.asarray(candidates).astype(np.int64)
    W_mu = np.ascontiguousarray(np.asarray(W_mu, dtype=np.float32))
    b_mu = np.asarray(b_mu, dtype=np.float32)
    emb16 = np.ascontiguousarray(
        np.asarray(emb_table, dtype=np.float32).astype(np.float16)
    )
    nc = _build_program_indirect()
    bias_tile = np.broadcast_to(b_mu.reshape(1, D), (128, D))
    in_maps = []
    for core in range(NCORES):
        sl = slice(core * B_LOC, (core + 1) * B_LOC)
        consts = np.concatenate([W_mu, x[sl].T, bias_tile], axis=1)
        # cidx[p, c*N + j] = candidates[core*B_LOC + c*128 + p, j]
        cc = candidates[sl].reshape(NBLK, 128, N).transpose(1, 0, 2).reshape(128, NBLK * N)
        in_maps.append(
            {
                "consts": np.ascontiguousarray(consts, dtype=np.float32),
                "cidx": np.ascontiguousarray(cc.astype(np.uint32)),
                "emb": emb16,
            }
        )
    return nc, in_maps


def assemble_indirect(results):
    logits = np.zeros((B, N), dtype=np.float32)
    for core in range(len(results)):
        oc = results[core]["out"]  # [128, NBLK*N]
        rs = slice(core * B_LOC, (core + 1) * B_LOC)
        logits[rs] = oc.reshape(128, NBLK, N).transpose(1, 0, 2).reshape(B_LOC, N)
    return logits


_BASES_HACK = None  # set by _build_program; window base row offsets into emb


def _build_program(bases, g_list, out_cols, tot_cols):
    global _BASES_HACK
    key = (tuple(bases), tuple(g_list), ACT_FUNC, REPEATS, FOLD_STOP, SINGLE_PACKET)
    if key in _program_cache:
        return _program_cache[key]
    nc = bacc.Bacc(
        "TRN2",
        target_bir_lowering=False,
        debug=False,
        enable_asserts=False,
        num_devices=NCORES,
        num_swdge_queues=4,
    )
    total_words = sum(NBLK * g * 128 // 16 for g in g_list)
    consts = nc.dram_tensor("consts", (128, CONST_COLS), _f32, kind="ExternalInput").ap()
    gidx = nc.dram_tensor("gidx", (128, total_words), _i16, kind="ExternalInput").ap()
    emb = nc.dram_tensor("emb", (V, D), _f16, kind="ExternalInput").ap()
    out = nc.dram_tensor("out", (128, NBLK * tot_cols), _f32, kind="ExternalOutput").ap()
    _BASES_HACK = bases
    with tile.TileContext(nc) as tc:
        _kernel_body(tc, consts, gidx, emb, out, g_list, out_cols, tot_cols)
    nc.finalize()
    _program_cache[key] = nc
    return nc


# --------------------------------------------------------------------------
# host packing
# --------------------------------------------------------------------------

def prepare(x, candidates, W_mu, b_mu, mu_bias, emb_table):
    x = np.asarray(x, dtype=np.float32)
    candidates = np.asarray(candidates).astype(np.int64)
    W_mu = np.ascontiguousarray(np.asarray(W_mu, dtype=np.float32))
    b_mu = np.asarray(b_mu, dtype=np.float32)
    emb16 = np.ascontiguousarray(np.asarray(emb_table, dtype=np.float32).astype(np.float16))

    order = np.argsort(candidates, axis=1, kind="stable")    # [B, N]
    sc = np.take_along_axis(candidates, order, axis=1)       # sorted values

    best = None
    for nw in (4, 5):  # nw=6's 24 instructions regressed on HW (757us)
        res = _assign_windows(sc, nw)
        if res is None:
            continue
        bases, loads, G = res
        g_list = [int(loads[:, k].max()) for k in range(nw)]
        tot = sum(g_list)
        if best is None or tot < best[0]:
            best = (tot, bases, loads, g_list)
    assert best is not None, "window assignment failed"
    tot_cols, bases, loads, g_list = best
    nw = len(g_list)
    out_cols = np.concatenate([[0], np.cumsum(g_list)])[:nw].tolist()

    # col_map: for batch row b, sorted-candidate j -> output column within the
    # row's block: col = off_k + s  (s = j - start of window k's run)
    starts = np.concatenate(
        [np.zeros((B, 1), dtype=np.int64), np.cumsum(loads, axis=1)], axis=1
    )  # [B, nw+1]; run k = sorted positions [starts[k], starts[k+1])
    col_sorted = np.empty((B, N), dtype=np.int64)
    win_of = np.empty((B, N), dtype=np.int64)
    for k in range(nw):
        s0 = starts[:, k]
        s1 = starts[:, k + 1]
        j = np.arange(N)[None, :]
        m = (j >= s0[:, None]) & (j < s1[:, None])
        col_sorted[m] = (out_cols[k] + (j - s0[:, None]))[m]
        win_of[m] = k
    # map back to original candidate order: col_map[b, order[b,j]] = col_sorted[b,j]
    col_map = np.empty((B, N), dtype=np.int64)
    np.put_along_axis(col_map, order, col_sorted, axis=1)

    # per-core packed gather indices
    gidx_tiles = []
    total_words = sum(NBLK * g * 128 // 16 for g in g_list)
    for core in range(NCORES):
        rs = slice(core * B_LOC, (core + 1) * B_LOC)
        sc_c = sc[rs]
        starts_c = starts[rs]
        words = np.empty((16, total_words), dtype=np.uint16)
        woff = 0
        for k in range(nw):
            g_k = g_list[k]
            vals = np.zeros((B_LOC, g_k), dtype=np.uint16)  # pad -> idx 0
            ld = (starts_c[:, k + 1] - starts_c[:, k]).astype(np.int64)
            # gather run k of each row
            j = np.arange(g_k)[None, :]
            src = starts_c[:, k][:, None] + j
            m = j < ld[:, None]
            picked = np.take_along_axis(sc_c, np.minimum(src, N - 1), axis=1)
            vals[m] = (picked - bases[k]).astype(np.uint16)[m]
            # slot grid: flat j = (c*g_k + t)*128 + p ; vals[c*128+p, t]
            v = vals.reshape(NBLK, 128, g_k).transpose(0, 2, 1).reshape(-1)
            nwords = v.size // 16
            words[:, woff : woff + nwords] = v.reshape(nwords, 16).T
            woff += nwords
        gidx_tiles.append(np.tile(words, (8, 1)).view(np.int16))

    # sanity: indices in range for every window
    for k in range(nw):
        m = win_of == k
        if m.any():
            assert int(sc[m].min()) >= bases[k], (k, int(sc[m].min()), bases[k])
            assert int(sc[m].max()) - bases[k] <= 32767, (k, int(sc[m].max()), bases[k])

    nc = _build_program(tuple(bases), g_list, out_cols, tot_cols)

    bias_tile = np.broadcast_to(b_mu.reshape(1, D), (128, D))
    in_maps = []
    for core in range(NCORES):
        sl = slice(core * B_LOC, (core + 1) * B_LOC)
        consts = np.concatenate([W_mu, x[sl].T, bias_tile], axis=1)
        in_maps.append(
            {
                "consts": np.ascontiguousarray(consts, dtype=np.float32),
                "gidx": np.ascontiguousarray(gidx_tiles[core]),
                "emb": emb16,
            }
        )
    return nc, in_maps, col_map, tot_cols


def assemble(results, col_map, tot_cols):
    logits = np.zeros((B, N), dtype=np.float32)
    for core in range(len(results)):
        oc = results[core]["out"]  # [128, NBLK * tot_cols]
        oc = oc.reshape(128, NBLK, tot_cols).transpose(1, 0, 2).reshape(B_LOC, tot_cols)
        rs = slice(core * B_LOC, (core + 1) * B_LOC)
        logits[rs] = np.take_along_axis(oc, col_map[rs], axis=1)
    return logits


def kernel(x, candidates, W_mu, b_mu, mu_bias, emb_table):
    global LAST_RESULTS
    candidates = np.asarray(candidates).astype(np.int64)
    mu_bias = np.asarray(mu_bias, dtype=np.float32)
    ncores_run = int(os.environ.get("KERNEL_CORES", NCORES))
    if USE_INDIRECT:
        nc, in_maps = prepare_indirect(
            x, candidates, W_mu, b_mu, mu_bias, emb_table
        )
        res = run_bass_kernel_spmd(
            nc, in_maps[:ncores_run], core_ids=list(range(ncores_run)), trace=TRACE
        )
        LAST_RESULTS = res
        logits = assemble_indirect(res.results)
    else:
        nc, in_maps, col_map, tot_cols = prepare(
            x, candidates, W_mu, b_mu, mu_bias, emb_table
        )
        res = run_bass_kernel_spmd(
            nc, in_maps[:ncores_run], core_ids=list(range(ncores_run)), trace=TRACE
        )
        LAST_RESULTS = res
        logits = assemble(res.results, col_map, tot_cols)
    if np.any(mu_bias):
        logits = logits + mu_bias[candidates]
    return np.ascontiguousarray(logits.astype(np.float32))
